# revision 1
# baseline (speedup 1.0000x reference)
"""AttnBlock1D (BN + single-head 1x1-conv attention + residual) on 8 TRN2 cores.

Contract: kernel(**inputs) takes the FULL inputs from setup_inputs() and
returns the FULL output [4, 256, 4096] f32. Measured ~238 us HW exec,
norm-relative error ~5e-4 (absmax ~6e-3 on an output scale of ~5).

Sharding: 8 cores = 4 samples x 2 query-halves (data-parallel over B,
attention split over queries). Core i handles sample b = i // 2 and
queries [qh*2048, (qh+1)*2048), qh = i % 2. The host rolls x[b] along L
so each core's queries are the FIRST 2048 columns -- attention is
permutation-invariant over keys, so k/v built from the rolled layout give
identical softmax results; the SPMD program needs no per-core constants.

BatchNorm stats are computed locally on every core -- NO collective. Any
cross-core sync puts the NEFF start skew across the 8 cores (33-65 us,
run-variable) onto the measured span; recomputing stats locally costs
~35 us, fully overlapped with input DMA, and is deterministic. Inputs per
core: x (fp32, rolled; residual only, DMA'd last), x16 (bf16 of the same,
compute + own-sample stats), xs (fp8-e4m3 copy of the other 3 samples,
stats only; quantization shifts the batch stats by ~1e-4 relative).
Stats are split across engines so they chase the DMA stream: most
512-chunks of each [128, 4096] tile go through DVE bn_stats/bn_aggr
(own tile 4, xs tiles 6), the rest through Scalar-engine Copy/Square
activations with accum_out in 1024-wide pieces; a few trivial matmuls
paced by the stats keep the PE from idling. Totals are
combined into biased mean/var exactly (all bn_stats packs have equal
counts; the accumulated sums are added via scalar_tensor_tensor).

The BN affine (h = x*a + d, a = gamma*rsqrt(var+eps), d = beta - mean*a)
is folded into the projections on-device: effective biases b + w @ d via
N=1 matvec matmuls on the raw weights, then wT is scaled in place by a
per input channel (Scalar-engine Copy with a per-partition scale, so the
busy DVE stays off the critical path). So the q/k/v matmuls read x16 directly and the only
stats-dependent serial work is ~3 us of small ops. The v-path constant
(wv @ d + bv) is softmax-invariant (rows of softmax sum to 1) and folds
into the output projection bias: bpe = bp + wp @ bv (host) + wp @ (wv @ d)
(device).

Matmuls run in bf16 (1 cycle/row on the PE at 2.4 GHz; fp16 measured 2x
slower in-kernel), fp32 PSUM accumulation. Attention scores are computed
transposed, ST[j, i] = sum_c k[c, j] q[c, i], so after exp (Scalar
engine, scale=1/16 folded in, no max-subtraction needed -- scores are
~N(0,1) so exp is safe in fp32) the probabilities land with j (keys) on
the partition axis, which the AV matmul contracts natively -- no
transposes anywhere. v is produced transposed ([l, o] tiles) by swapping
matmul operands. The softmax denominator comes from a ones[128,128]
stationary matmul over the same pT tiles, which also broadcasts it
across all partitions; reciprocal_approx_fast + one tensor_mul per
channel-half normalizes straight out of PSUM. Queries are processed in 5
chunks (3x512 + 2x256; the smaller final chunks shorten the serial
epilogue tail) with double-buffered probability tiles so score matmuls
of chunk n+1 overlap the AV/projection of chunk n.
"""

import os

import numpy as np
import ml_dtypes

import concourse.bass as bass
import concourse.mybir as mybir
import concourse.tile as tile
from concourse import bacc
from concourse import bass_utils

F32 = mybir.dt.float32
BF16 = mybir.dt.bfloat16
F16 = mybir.dt.float16

N_CORES = 8
B, C, L = 4, 256, 4096
M = L // 2          # queries per core
EPS = 1e-5
SCALE = 1.0 / 16.0  # C ** -0.5

NCHUNK = 4          # query chunks per core
CH = M // NCHUNK    # 512 queries per chunk
NJT = L // 128      # 32 key tiles
AF = mybir.ActivationFunctionType

LAST_EXEC_NS = None
_COMPILED = None


def _build():
    nc = bacc.Bacc("TRN2", target_bir_lowering=False, debug=False,
                   num_devices=N_CORES)

    x_d = nc.dram_tensor("x", [C, L], F32, kind="ExternalInput")
    x16_d = nc.dram_tensor("x16", [C, L], BF16, kind="ExternalInput")
    xs_d = nc.dram_tensor("xs", [(B - 1) * C, L], mybir.dt.float8e4, kind="ExternalInput")
    wq_d = nc.dram_tensor("wqT", [C, C], BF16, kind="ExternalInput")
    wk_d = nc.dram_tensor("wkT", [C, C], BF16, kind="ExternalInput")
    wv_d = nc.dram_tensor("wvT", [C, C], BF16, kind="ExternalInput")
    wp_d = nc.dram_tensor("wpT", [C, C], BF16, kind="ExternalInput")
    bq_d = nc.dram_tensor("bq", [C, 1], F32, kind="ExternalInput")
    bk_d = nc.dram_tensor("bk", [C, 1], F32, kind="ExternalInput")
    bp_d = nc.dram_tensor("bpe", [C, 1], F32, kind="ExternalInput")
    gam_d = nc.dram_tensor("gamma", [C, 1], F32, kind="ExternalInput")
    bet_d = nc.dram_tensor("beta", [C, 1], F32, kind="ExternalInput")
    out_d = nc.dram_tensor("out", [C, M], F32, kind="ExternalOutput")

    with tile.TileContext(nc) as tc:
        with (
            tc.tile_pool(name="big", bufs=1) as big,
            tc.tile_pool(name="pt", bufs=2) as ptp,
            tc.tile_pool(name="small", bufs=2) as sm,
            tc.tile_pool(name="eps", bufs=3) as epi,
            tc.tile_pool(name="ps_s", bufs=2, space="PSUM") as ps_s,
            tc.tile_pool(name="ps_acc", bufs=1, space="PSUM") as ps_acc,
            tc.tile_pool(name="ps_o", bufs=1, space="PSUM") as ps_o,
        ):
            # ---- DMA: x16 first (stats+compute), xs, weights; f32 x last
            x16_t = [big.tile([128, L], BF16, name=f"x16_{h}")
                     for h in range(2)]
            for h in range(2):
                nc.sync.dma_start(x16_t[h][:], x16_d[h * 128:(h + 1) * 128, :])

            vecs = {}
            for nm, d in (("bq", bq_d), ("bk", bk_d), ("bpe", bp_d),
                          ("gam", gam_d), ("bet", bet_d)):
                vecs[nm] = [big.tile([128, 1], F32, name=f"{nm}{h}")
                            for h in range(2)]
                for h in range(2):
                    nc.sync.dma_start(vecs[nm][h][:],
                                      d[h * 128:(h + 1) * 128, :])

            ones_t = big.tile([128, 128], BF16, name="ones")
            nc.vector.memset(ones_t[:], 1.0)

            # ------- BN stats, split across DVE (bn_stats) and ACT --------
            # 8 stat tiles per core: own sample (x16, bf16) + 3 other
            # samples (xs, fp16), each [128, 4096] per channel-half.
            # Per tile: chunks 0-4 go through DVE bn_stats, chunks 5-7
            # through ACT Square/Copy accumulations -- balances the two
            # engines so stats finish right behind the DMA stream.
            NDVE = 22             # bn_stats packs per channel-half
            s6_dve = [sm.tile([128, NDVE * 6], F32, name=f"s6d{h}")
                      for h in range(2)]
            asum = [sm.tile([128, 5], F32, name=f"asum{h}") for h in range(2)]
            assq = [sm.tile([128, 5], F32, name=f"assq{h}") for h in range(2)]
            _pk = [0, 0]
            _ac = [0, 0]

            def stat_tile(tile_ap, h, ndve):
                for i in range(ndve):
                    p = _pk[h]; _pk[h] += 1
                    nc.vector.bn_stats(
                        s6_dve[h][:, p * 6:(p + 1) * 6],
                        tile_ap[:, i * 512:(i + 1) * 512])
                    if i % 2 == 1:
                        # PE activity paced by the stats stream (HAM warmth)
                        wps = ps_s.tile([128, 12], F32, tag="s",
                                        name=f"wbn{h}_{p}")
                        nc.tensor.matmul(
                            wps[:], ones_t[:],
                            s6_dve[h][:, p * 6:(p + 1) * 6].bitcast(BF16),
                            start=True, stop=True)
                pos = ndve * 512
                while pos < L:
                    w = min(1024, L - pos)
                    col = _ac[h]; _ac[h] += 1
                    cs = slice(pos, pos + w)
                    scr0 = sm.tile([128, 1024], BF16, tag="scr", bufs=4,
                                   name=f"scrS{h}_{col}")
                    nc.scalar.activation(scr0[:, :w], tile_ap[:, cs], AF.Copy,
                                         accum_out=asum[h][:, col:col + 1])
                    scr1 = sm.tile([128, 1024], BF16, tag="scr", bufs=4,
                                   name=f"scrQ{h}_{col}")
                    nc.scalar.activation(scr1[:, :w], tile_ap[:, cs], AF.Square,
                                         accum_out=assq[h][:, col:col + 1])
                    # HAM warmup: trivial matmul paced by the stats stream
                    wp_ps = ps_s.tile([128, 512], F32, tag="s",
                                      name=f"warm{h}_{col}")
                    nc.tensor.matmul(wp_ps[:], ones_t[:], scr1[:, 0:512],
                                     start=True, stop=True)
                    pos += w

            for h in range(2):
                stat_tile(x16_t[h][:], h, 4)

            for s in range(B - 1):
                for h in range(2):
                    xs_t = sm.tile([128, L], mybir.dt.float8e4, tag="xs", bufs=3,
                                   name=f"xs{s}_{h}")
                    row0 = s * C + h * 128
                    for q2 in range(2):
                        qs = slice(q2 * 2048, (q2 + 1) * 2048)
                        nc.sync.dma_start(xs_t[:, qs],
                                          xs_d[row0:row0 + 128, qs])
                    stat_tile(xs_t[:], h, 6)

            # weights stream in behind the stats inputs
            w_t = {}
            for nm, d in (("q", wq_d), ("k", wk_d), ("v", wv_d), ("p", wp_d)):
                w_t[nm] = [big.tile([128, C], BF16, name=f"w{nm}{h}")
                           for h in range(2)]
                for h in range(2):
                    nc.sync.dma_start(w_t[nm][h][:],
                                      d[h * 128:(h + 1) * 128, :])

            # f32 x arrives late; only the epilogue residual reads it
            x_t = [big.tile([128, L], F32, name=f"x{h}") for h in range(2)]
            for h in range(2):
                nc.sync.dma_start(x_t[h][:], x_d[h * 128:(h + 1) * 128, :])

            # ------- combine stats -> a (scale), d (shift) per channel ----
            ND = NDVE * 512           # elements covered by the DVE packs
            NT = B * L
            a_t, d_t = [], []
            for h in range(2):
                s2 = sm.tile([128, 2], F32, name=f"s2_{h}")
                nc.vector.bn_aggr(s2[:], s6_dve[h][:])
                sa = sm.tile([128, 1], F32, name=f"sa{h}")
                nc.vector.reduce_sum(sa[:], asum[h][:], axis=mybir.AxisListType.X)
                qa = sm.tile([128, 1], F32, name=f"qa{h}")
                nc.vector.reduce_sum(qa[:], assq[h][:], axis=mybir.AxisListType.X)
                tot = sm.tile([128, 1], F32, name=f"tot{h}")
                nc.vector.scalar_tensor_tensor(
                    out=tot[:], in0=s2[:, 0:1], scalar=float(ND), in1=sa[:],
                    op0=mybir.AluOpType.mult, op1=mybir.AluOpType.add)
                mo2 = sm.tile([128, 1], F32, name=f"mo2{h}")
                nc.vector.tensor_mul(mo2[:], s2[:, 0:1], s2[:, 0:1])
                e2o = sm.tile([128, 1], F32, name=f"e2o{h}")
                nc.vector.tensor_add(e2o[:], s2[:, 1:2], mo2[:])
                totq = sm.tile([128, 1], F32, name=f"totq{h}")
                nc.vector.scalar_tensor_tensor(
                    out=totq[:], in0=e2o[:], scalar=float(ND), in1=qa[:],
                    op0=mybir.AluOpType.mult, op1=mybir.AluOpType.add)
                # ngm = -mean;  ge2p = E[x^2] + EPS;  var = ge2p - ngm^2
                ngm = sm.tile([128, 1], F32, name=f"ngm{h}")
                nc.vector.tensor_scalar_mul(ngm[:], tot[:], -1.0 / NT)
                ge2p = sm.tile([128, 1], F32, name=f"ge2p{h}")
                nc.vector.tensor_scalar(
                    out=ge2p[:], in0=totq[:], scalar1=1.0 / NT, scalar2=EPS,
                    op0=mybir.AluOpType.mult, op1=mybir.AluOpType.add)
                var = sm.tile([128, 1], F32, name=f"var{h}")
                nc.vector.scalar_tensor_tensor(
                    out=var[:], in0=ngm[:], scalar=ngm[:], in1=ge2p[:],
                    op0=mybir.AluOpType.mult, op1=mybir.AluOpType.subtract)
                nc.vector.tensor_scalar_mul(var[:], var[:], -1.0)
                sd = sm.tile([128, 1], F32, name=f"sd{h}")
                nc.scalar.activation(sd[:], var[:], AF.Sqrt)
                rs = sm.tile([128, 1], F32, name=f"rs{h}")
                nc.vector.reciprocal(rs[:], sd[:])
                a = sm.tile([128, 1], F32, name=f"a{h}")
                nc.vector.tensor_mul(a[:], rs[:], vecs["gam"][h][:])
                dd = sm.tile([128, 1], F32, name=f"d{h}")
                nc.vector.scalar_tensor_tensor(
                    out=dd[:], in0=a[:], scalar=ngm[:], in1=vecs["bet"][h][:],
                    op0=mybir.AluOpType.mult, op1=mybir.AluOpType.add)
                a_t.append(a)
                d_t.append(dd)

            # ------- fold BN affine into weights + effective biases -------
            # b*_eff = w @ d + b uses the RAW weights (tiny matvecs), then
            # w is scaled IN PLACE: w[c, o] *= a[c].
            # d as a bf16 [128,1] for the tiny matvecs
            d16 = [sm.tile([128, 1], BF16, name=f"d16_{h}") for h in range(2)]
            for h in range(2):
                nc.vector.tensor_copy(d16[h][:], d_t[h][:])

            def matvec(wtiles, rhs16, name):
                """out[o] = sum_c w[o, c] * rhs[c] as [2][128, 1] sbuf f32"""
                outs = []
                for oh in range(2):
                    ps = ps_s.tile([128, 1], F32, tag="s", name=f"mv_{name}{oh}")
                    for ch in range(2):
                        nc.tensor.matmul(
                            ps[:],
                            wtiles[ch][:, oh * 128:(oh + 1) * 128],
                            rhs16[ch][:],
                            start=(ch == 0), stop=(ch == 1),
                        )
                    o = sm.tile([128, 1], F32, name=f"mvo_{name}{oh}")
                    nc.vector.tensor_copy(o[:], ps[:])
                    outs.append(o)
                return outs

            wqd = matvec(w_t["q"], d16, "q")
            wkd = matvec(w_t["k"], d16, "k")
            wvd = matvec(w_t["v"], d16, "v")
            bq_e, bk_e = [], []
            for oh in range(2):
                t = sm.tile([128, 1], F32, name=f"bqe{oh}")
                nc.vector.tensor_add(t[:], wqd[oh][:], vecs["bq"][oh][:])
                bq_e.append(t)
                t = sm.tile([128, 1], F32, name=f"bke{oh}")
                nc.vector.tensor_add(t[:], wkd[oh][:], vecs["bk"][oh][:])
                bk_e.append(t)
            # bpe_eff = bpe + wp @ (wv @ d)
            wvd16 = [sm.tile([128, 1], BF16, name=f"wvd16_{h}")
                     for h in range(2)]
            for h in range(2):
                nc.vector.tensor_copy(wvd16[h][:], wvd[h][:])
            wpwvd = matvec(w_t["p"], wvd16, "p")
            bp_e = []
            for oh in range(2):
                t = sm.tile([128, 1], F32, name=f"bpe_e{oh}")
                nc.vector.tensor_add(t[:], wpwvd[oh][:], vecs["bpe"][oh][:])
                bp_e.append(t)

            for nm in ("q", "k", "v"):
                for h in range(2):
                    nc.scalar.activation(
                        w_t[nm][h][:], w_t[nm][h][:], AF.Copy,
                        scale=a_t[h][:])

            # ---------------- projections (read x16 directly) -------------
            q_t = [big.tile([128, M], BF16, name=f"q{h}") for h in range(2)]
            k_t = [big.tile([128, L], BF16, name=f"k{h}") for h in range(2)]
            vT_t = big.tile([128, NJT * 256], BF16, name="vT")

            for lt in range(NJT):
                ps = ps_s.tile([128, 512], F32, tag="s", name="ps_v")
                for ch in range(2):
                    nc.tensor.matmul(
                        ps[:, 0:256],
                        x16_t[ch][:, lt * 128:(lt + 1) * 128],
                        w_t["v"][ch][:],
                        start=(ch == 0), stop=(ch == 1),
                    )
                nc.vector.tensor_copy(
                    vT_t[:, lt * 256:(lt + 1) * 256], ps[:, 0:256])

            for oh in range(2):
                for it in range(M // 512):
                    ps = ps_s.tile([128, 512], F32, tag="s", name="ps_q")
                    for ch in range(2):
                        nc.tensor.matmul(
                            ps[:],
                            w_t["q"][ch][:, oh * 128:(oh + 1) * 128],
                            x16_t[ch][:, it * 512:(it + 1) * 512],
                            start=(ch == 0), stop=(ch == 1),
                        )
                    nc.vector.tensor_scalar_add(
                        q_t[oh][:, it * 512:(it + 1) * 512], ps[:],
                        bq_e[oh][:])

            for oh in range(2):
                for it in range(L // 512):
                    ps = ps_s.tile([128, 512], F32, tag="s", name="ps_k")
                    for ch in range(2):
                        nc.tensor.matmul(
                            ps[:],
                            w_t["k"][ch][:, oh * 128:(oh + 1) * 128],
                            x16_t[ch][:, it * 512:(it + 1) * 512],
                            start=(ch == 0), stop=(ch == 1),
                        )
                    nc.vector.tensor_scalar_add(
                        k_t[oh][:, it * 512:(it + 1) * 512], ps[:],
                        bk_e[oh][:])

            # ---------------- attention, chunk by chunk ----------------
            chunks = [(0, 512), (512, 512), (1024, 512),
                      (1536, 256), (1792, 256)]
            for cn, (i0, chw) in enumerate(chunks):
                pT = ptp.tile([128, NJT * chw], BF16, tag="pT", name=f"pT{cn}")
                for jp in range(NJT // 2):
                    ps = ps_s.tile([128, 2 * chw], F32, tag="s", name="ps_sc")
                    for half in range(2):
                        jt = jp * 2 + half
                        for ch in range(2):
                            nc.tensor.matmul(
                                ps[:, half * chw:(half + 1) * chw],
                                k_t[ch][:, jt * 128:(jt + 1) * 128],
                                q_t[ch][:, i0:i0 + chw],
                                start=(ch == 0), stop=(ch == 1),
                            )
                    nc.scalar.activation(
                        pT[:, jp * 2 * chw:(jp + 1) * 2 * chw], ps[:],
                        AF.Exp, scale=SCALE)

                ps_av = [ps_acc.tile([128, chw], F32, tag=f"av{ch}",
                                     name=f"av{ch}_{cn}") for ch in range(2)]
                ps_den = ps_acc.tile([128, chw], F32, tag="den",
                                     name=f"den{cn}")
                for jt in range(NJT):
                    pslice = pT[:, jt * chw:(jt + 1) * chw]
                    for ch in range(2):
                        nc.tensor.matmul(
                            ps_av[ch][:],
                            vT_t[:, jt * 256 + ch * 128:jt * 256 + (ch + 1) * 128],
                            pslice,
                            start=(jt == 0), stop=(jt == NJT - 1),
                        )
                    nc.tensor.matmul(
                        ps_den[:], ones_t[:], pslice,
                        start=(jt == 0), stop=(jt == NJT - 1),
                    )

                rec = epi.tile([128, chw], F32, tag="rec", name=f"rec{cn}")
                nc.vector.reciprocal_approx_fast(rec[:], ps_den[:])

                at_t = []
                for ch in range(2):
                    at = epi.tile([128, chw], BF16, tag=f"at{ch}",
                                  name=f"at{ch}_{cn}")
                    nc.vector.tensor_mul(at[:], ps_av[ch][:], rec[:])
                    at_t.append(at)

                for oh in range(2):
                    ps = ps_o.tile([128, chw], F32, tag="o", name=f"po{oh}_{cn}")
                    for ch in range(2):
                        nc.tensor.matmul(
                            ps[:],
                            w_t["p"][ch][:, oh * 128:(oh + 1) * 128],
                            at_t[ch][:],
                            start=(ch == 0), stop=(ch == 1),
                        )
                    res = epi.tile([128, chw], F32, tag="res",
                                   name=f"res{oh}_{cn}")
                    nc.vector.scalar_tensor_tensor(
                        out=res[:], in0=ps[:], scalar=bp_e[oh][:],
                        in1=x_t[oh][:, i0:i0 + chw],
                        op0=mybir.AluOpType.add, op1=mybir.AluOpType.add,
                    )
                    nc.sync.dma_start(
                        out_d[oh * 128:(oh + 1) * 128, i0:i0 + chw], res[:])

    nc.compile()
    return nc


def kernel(x, gamma, beta, wq, bq, wk, bk, wv, bv, wp, bp):
    global _COMPILED, LAST_EXEC_NS
    x = np.asarray(x, np.float32)
    if _COMPILED is None:
        _COMPILED = _build()
    nc = _COMPILED

    common = {
        "wqT": np.ascontiguousarray(np.asarray(wq, np.float32).T).astype(ml_dtypes.bfloat16),
        "wkT": np.ascontiguousarray(np.asarray(wk, np.float32).T).astype(ml_dtypes.bfloat16),
        "wvT": np.ascontiguousarray(np.asarray(wv, np.float32).T).astype(ml_dtypes.bfloat16),
        "wpT": np.ascontiguousarray(np.asarray(wp, np.float32).T).astype(ml_dtypes.bfloat16),
        "bq": np.asarray(bq, np.float32).reshape(C, 1),
        "bk": np.asarray(bk, np.float32).reshape(C, 1),
        "bpe": (np.asarray(bp, np.float32)
                + np.asarray(wp, np.float32) @ np.asarray(bv, np.float32)
                ).reshape(C, 1),
        "gamma": np.asarray(gamma, np.float32).reshape(C, 1),
        "beta": np.asarray(beta, np.float32).reshape(C, 1),
    }

    x16 = [np.ascontiguousarray(x[b]).astype(ml_dtypes.float8_e4m3) for b in range(B)]

    in_maps = []
    for core in range(N_CORES):
        b, qh = core // 2, core % 2
        xb = x[b]
        if qh:
            xb = np.ascontiguousarray(np.roll(xb, -M, axis=1))
        others = np.concatenate([x16[s] for s in range(B) if s != b])
        in_maps.append({"x": xb, "x16": xb.astype(ml_dtypes.bfloat16),
                        "xs": others, **common})

    trace = os.environ.get("BASS_KERNEL_TRACE", "") == "1"
    res = bass_utils.run_bass_kernel_spmd(
        nc, in_maps, core_ids=list(range(N_CORES)), trace=trace)
    LAST_EXEC_NS = res.exec_time_ns

    out = np.empty((B, C, L), np.float32)
    for core in range(N_CORES):
        b, qh = core // 2, core % 2
        out[b, :, qh * M:(qh + 1) * M] = res.results[core]["out"]
    return out



# revision 5
# speedup vs baseline: 1.4502x; 1.4502x over previous
"""AttnBlock1D (BN + single-head 1x1-conv attention + residual) on 8 TRN2 cores.

Contract: kernel(**inputs) takes the FULL inputs from setup_inputs() and
returns the FULL output [4, 256, 4096] f32.

Sharding: 8 cores = 4 samples x 2 query-halves (data-parallel over B,
attention split over queries). Core i handles sample b = i // 2 and
queries [qh*2048, (qh+1)*2048), qh = i % 2. The host rolls x[b] along L
so each core's queries are the FIRST 2048 columns -- attention is
permutation-invariant over keys, so k/v built from the rolled layout give
identical softmax results; the SPMD program needs no per-core constants.

BatchNorm stats are computed locally on every core -- NO collective (any
cross-core sync puts the NEFF start skew onto the measured span). Inputs
per core: x (fp32, rolled; residual only, DMA'd last), x16 (bf16 of the
same, compute + own-sample stats), xs (fp8-e4m3 copy of the other 3
samples, stats only). Stats are split across engines so they chase the
DMA stream: DVE bn_stats/bn_aggr for most 512-chunks, Scalar-engine
Copy/Square activations with accum_out for the rest; trivial matmuls
paced by the stats keep the PE warm. Totals combine into biased mean/var
exactly. rsqrt(var+eps) is computed as exp(-0.5*ln(var+eps)) so every
ACT function in the kernel (Copy/Square/Ln/Exp) lives in the single
natural_log_exp_and_others table -- zero mid-kernel ACT table loads.

The BN affine is folded into the projections on-device (weights scaled
in place by a = gamma*rsqrt(var+eps); effective biases via N=1 matvecs).
The k bias drops entirely: softmax is shift-invariant per query, so
S'[j,i] = (q_i + bq) . k_j gives identical probabilities. The v-path
constant folds into the output projection bias.

Attention runs in fp8-e4m3 with DoubleRow matmuls (contract 256 in one
instruction, ~2.2x bf16 rate measured): q/k projections (bf16 matmuls on
x16) write fp8 q8/k8 laid out [128, 2ch, cols]; scores per key tile are
one DoubleRow matmul; exp runs on ACT with scale=1/16 and bias=-3 folded
in (max scaled score ~7.9 overflows e4m3's 448 without the shift, which
softmax cancels), writing fp8 probabilities with keys on partitions; AV
and the ones-matmul denominator (broadcast across partitions for free)
also contract via DoubleRow from the same pT tiles. Probabilities below
e4m3 subnormal range flush to zero -- their true softmax weight is
<1e-7 of the denominator. reciprocal_approx_fast + one tensor_mul per
channel-half normalizes straight out of PSUM; output projection runs
bf16. Queries are processed in 5 chunks (3x512 + 2x256) with
double-buffered probability tiles; chunk-0 scores are emitted before the
v projection so ACT exp starts as early as possible.
"""

import os

import numpy as np
import ml_dtypes

import concourse.bass as bass
import concourse.mybir as mybir
import concourse.tile as tile
from concourse import bacc
from concourse import bass_utils

F32 = mybir.dt.float32
BF16 = mybir.dt.bfloat16
F8 = mybir.dt.float8e4
DR = mybir.MatmulPerfMode.DoubleRow

N_CORES = 8
B, C, L = 4, 256, 4096
M = L // 2          # queries per core
EPS = 1e-5
SCALE = 1.0 / 16.0  # C ** -0.5
CSHIFT = 3.0        # exp bias: p = exp(s/16 - CSHIFT); cancels in softmax

NJT = L // 128      # 32 key tiles
NJP = NJT // 2      # 16 key-tile pairs (DoubleRow contracts 256 keys)
AF = mybir.ActivationFunctionType

LAST_EXEC_NS = None
_COMPILED = None


def _build():
    nc = bacc.Bacc("TRN2", target_bir_lowering=False, debug=False,
                   num_devices=N_CORES)

    x_d = nc.dram_tensor("x", [C, L], F32, kind="ExternalInput")
    x16_d = nc.dram_tensor("x16", [C, L], BF16, kind="ExternalInput")
    xs_d = nc.dram_tensor("xs", [(B - 1) * C, L], F8, kind="ExternalInput")
    wq_d = nc.dram_tensor("wqT", [C, C], BF16, kind="ExternalInput")
    wk_d = nc.dram_tensor("wkT", [C, C], BF16, kind="ExternalInput")
    wv_d = nc.dram_tensor("wvT", [C, C], BF16, kind="ExternalInput")
    wp_d = nc.dram_tensor("wpT", [C, C], BF16, kind="ExternalInput")
    bq_d = nc.dram_tensor("bq", [C, 1], F32, kind="ExternalInput")
    bp_d = nc.dram_tensor("bpe", [C, 1], F32, kind="ExternalInput")
    gam_d = nc.dram_tensor("gamma", [C, 1], F32, kind="ExternalInput")
    bet_d = nc.dram_tensor("beta", [C, 1], F32, kind="ExternalInput")
    out_d = nc.dram_tensor("out", [C, M], F32, kind="ExternalOutput")

    with tile.TileContext(nc) as tc:
        with (
            tc.tile_pool(name="big", bufs=1) as big,
            tc.tile_pool(name="pt", bufs=2) as ptp,
            tc.tile_pool(name="small", bufs=2) as sm,
            tc.tile_pool(name="eps", bufs=3) as epi,
            tc.tile_pool(name="ps_s", bufs=2, space="PSUM") as ps_s,
            tc.tile_pool(name="ps_acc", bufs=1, space="PSUM") as ps_acc,
            tc.tile_pool(name="ps_o", bufs=1, space="PSUM") as ps_o,
        ):
            # ---- DMA: x16 first (stats+compute), xs, weights; f32 x last
            x16_t = [big.tile([128, L], BF16, name=f"x16_{h}")
                     for h in range(2)]
            for h in range(2):
                nc.sync.dma_start(x16_t[h][:], x16_d[h * 128:(h + 1) * 128, :])

            vecs = {}
            for nm, d in (("bq", bq_d), ("bpe", bp_d),
                          ("gam", gam_d), ("bet", bet_d)):
                vecs[nm] = [big.tile([128, 1], F32, name=f"{nm}{h}")
                            for h in range(2)]
                for h in range(2):
                    nc.sync.dma_start(vecs[nm][h][:],
                                      d[h * 128:(h + 1) * 128, :])

            ones_t = big.tile([128, 128], BF16, name="ones")
            nc.vector.memset(ones_t[:], 1.0)
            ones8 = big.tile([128, 2, 128], F8, name="ones8")
            nc.vector.memset(ones8[:], 1.0)
            csh = big.tile([128, 1], F32, name="csh")
            nc.vector.memset(csh[:], -CSHIFT)

            # ------- BN stats, split across DVE (bn_stats) and ACT --------
            # 8 stat tiles per core: own sample (x16, bf16) + 3 other
            # samples (xs, fp8), each [128, 4096] per channel-half.
            NDVE = 22             # bn_stats packs per channel-half
            s6_dve = [sm.tile([128, NDVE * 6], F32, name=f"s6d{h}")
                      for h in range(2)]
            asum = [sm.tile([128, 5], F32, name=f"asum{h}") for h in range(2)]
            assq = [sm.tile([128, 5], F32, name=f"assq{h}") for h in range(2)]
            _pk = [0, 0]
            _ac = [0, 0]

            def stat_tile(tile_ap, h, ndve):
                for i in range(ndve):
                    p = _pk[h]; _pk[h] += 1
                    nc.vector.bn_stats(
                        s6_dve[h][:, p * 6:(p + 1) * 6],
                        tile_ap[:, i * 512:(i + 1) * 512])
                    if i % 2 == 1:
                        # PE activity paced by the stats stream (HAM warmth)
                        wps = ps_s.tile([128, 12], F32, tag="s",
                                        name=f"wbn{h}_{p}")
                        nc.tensor.matmul(
                            wps[:], ones_t[:],
                            s6_dve[h][:, p * 6:(p + 1) * 6].bitcast(BF16),
                            start=True, stop=True)
                pos = ndve * 512
                while pos < L:
                    w = min(1024, L - pos)
                    col = _ac[h]; _ac[h] += 1
                    cs = slice(pos, pos + w)
                    scr0 = sm.tile([128, 1024], BF16, tag="scr", bufs=4,
                                   name=f"scrS{h}_{col}")
                    nc.scalar.activation(scr0[:, :w], tile_ap[:, cs], AF.Copy,
                                         accum_out=asum[h][:, col:col + 1])
                    scr1 = sm.tile([128, 1024], BF16, tag="scr", bufs=4,
                                   name=f"scrQ{h}_{col}")
                    nc.scalar.activation(scr1[:, :w], tile_ap[:, cs], AF.Square,
                                         accum_out=assq[h][:, col:col + 1])
                    # HAM warmup: trivial matmul paced by the stats stream
                    wp_ps = ps_s.tile([128, 512], F32, tag="s",
                                      name=f"warm{h}_{col}")
                    nc.tensor.matmul(wp_ps[:], ones_t[:], scr1[:, 0:512],
                                     start=True, stop=True)
                    pos += w

            for h in range(2):
                stat_tile(x16_t[h][:], h, 4)

            for s in range(B - 1):
                for h in range(2):
                    xs_t = sm.tile([128, L], F8, tag="xs", bufs=3,
                                   name=f"xs{s}_{h}")
                    row0 = s * C + h * 128
                    for q2 in range(2):
                        qs = slice(q2 * 2048, (q2 + 1) * 2048)
                        nc.sync.dma_start(xs_t[:, qs],
                                          xs_d[row0:row0 + 128, qs])
                    stat_tile(xs_t[:], h, 6)

            # weights stream in behind the stats inputs
            w_t = {}
            for nm, d in (("q", wq_d), ("k", wk_d), ("v", wv_d), ("p", wp_d)):
                w_t[nm] = [big.tile([128, C], BF16, name=f"w{nm}{h}")
                           for h in range(2)]
                for h in range(2):
                    nc.sync.dma_start(w_t[nm][h][:],
                                      d[h * 128:(h + 1) * 128, :])

            # f32 x arrives late; only the epilogue residual reads it
            x_t = [big.tile([128, L], F32, name=f"x{h}") for h in range(2)]
            for h in range(2):
                nc.sync.dma_start(x_t[h][:], x_d[h * 128:(h + 1) * 128, :])

            # ------- combine stats -> a (scale), d (shift) per channel ----
            ND = NDVE * 512           # elements covered by the DVE packs
            NT = B * L
            a_t, d_t = [], []
            for h in range(2):
                s2 = sm.tile([128, 2], F32, name=f"s2_{h}")
                nc.vector.bn_aggr(s2[:], s6_dve[h][:])
                sa = sm.tile([128, 1], F32, name=f"sa{h}")
                nc.vector.reduce_sum(sa[:], asum[h][:], axis=mybir.AxisListType.X)
                qa = sm.tile([128, 1], F32, name=f"qa{h}")
                nc.vector.reduce_sum(qa[:], assq[h][:], axis=mybir.AxisListType.X)
                tot = sm.tile([128, 1], F32, name=f"tot{h}")
                nc.vector.scalar_tensor_tensor(
                    out=tot[:], in0=s2[:, 0:1], scalar=float(ND), in1=sa[:],
                    op0=mybir.AluOpType.mult, op1=mybir.AluOpType.add)
                mo2 = sm.tile([128, 1], F32, name=f"mo2{h}")
                nc.vector.tensor_mul(mo2[:], s2[:, 0:1], s2[:, 0:1])
                e2o = sm.tile([128, 1], F32, name=f"e2o{h}")
                nc.vector.tensor_add(e2o[:], s2[:, 1:2], mo2[:])
                totq = sm.tile([128, 1], F32, name=f"totq{h}")
                nc.vector.scalar_tensor_tensor(
                    out=totq[:], in0=e2o[:], scalar=float(ND), in1=qa[:],
                    op0=mybir.AluOpType.mult, op1=mybir.AluOpType.add)
                # ngm = -mean;  ge2p = E[x^2] + EPS;  var = ge2p - ngm^2
                ngm = sm.tile([128, 1], F32, name=f"ngm{h}")
                nc.vector.tensor_scalar_mul(ngm[:], tot[:], -1.0 / NT)
                ge2p = sm.tile([128, 1], F32, name=f"ge2p{h}")
                nc.vector.tensor_scalar(
                    out=ge2p[:], in0=totq[:], scalar1=1.0 / NT, scalar2=EPS,
                    op0=mybir.AluOpType.mult, op1=mybir.AluOpType.add)
                var = sm.tile([128, 1], F32, name=f"var{h}")
                nc.vector.scalar_tensor_tensor(
                    out=var[:], in0=ngm[:], scalar=ngm[:], in1=ge2p[:],
                    op0=mybir.AluOpType.mult, op1=mybir.AluOpType.subtract)
                nc.vector.tensor_scalar_mul(var[:], var[:], -1.0)
                # rs = rsqrt(var) = exp(-0.5 * ln(var)): stays in the one
                # ACT table (natural_log_exp_and_others), no table switch
                lnv = sm.tile([128, 1], F32, name=f"lnv{h}")
                nc.scalar.activation(lnv[:], var[:], AF.Ln)
                rs = sm.tile([128, 1], F32, name=f"rs{h}")
                nc.scalar.activation(rs[:], lnv[:], AF.Exp, scale=-0.5)
                a = sm.tile([128, 1], F32, name=f"a{h}")
                nc.vector.tensor_mul(a[:], rs[:], vecs["gam"][h][:])
                dd = sm.tile([128, 1], F32, name=f"d{h}")
                nc.vector.scalar_tensor_tensor(
                    out=dd[:], in0=a[:], scalar=ngm[:], in1=vecs["bet"][h][:],
                    op0=mybir.AluOpType.mult, op1=mybir.AluOpType.add)
                a_t.append(a)
                d_t.append(dd)

            # ------- fold BN affine into weights + effective biases -------
            # b*_eff = w @ d + b uses the RAW weights (tiny matvecs), then
            # w is scaled IN PLACE: w[c, o] *= a[c].
            d16 = [sm.tile([128, 1], BF16, name=f"d16_{h}") for h in range(2)]
            for h in range(2):
                nc.vector.tensor_copy(d16[h][:], d_t[h][:])

            def matvec(wtiles, rhs16, name):
                """out[o] = sum_c w[o, c] * rhs[c] as [2][128, 1] sbuf f32"""
                outs = []
                for oh in range(2):
                    ps = ps_s.tile([128, 1], F32, tag="s", name=f"mv_{name}{oh}")
                    for ch in range(2):
                        nc.tensor.matmul(
                            ps[:],
                            wtiles[ch][:, oh * 128:(oh + 1) * 128],
                            rhs16[ch][:],
                            start=(ch == 0), stop=(ch == 1),
                        )
                    o = sm.tile([128, 1], F32, name=f"mvo_{name}{oh}")
                    nc.vector.tensor_copy(o[:], ps[:])
                    outs.append(o)
                return outs

            wqd = matvec(w_t["q"], d16, "q")
            wvd = matvec(w_t["v"], d16, "v")
            bq_e = []
            for oh in range(2):
                t = sm.tile([128, 1], F32, name=f"bqe{oh}")
                nc.vector.tensor_add(t[:], wqd[oh][:], vecs["bq"][oh][:])
                bq_e.append(t)
            # bpe_eff = bpe + wp @ (wv @ d)
            wvd16 = [sm.tile([128, 1], BF16, name=f"wvd16_{h}")
                     for h in range(2)]
            for h in range(2):
                nc.vector.tensor_copy(wvd16[h][:], wvd[h][:])
            wpwvd = matvec(w_t["p"], wvd16, "p")
            bp_e = []
            for oh in range(2):
                t = sm.tile([128, 1], F32, name=f"bpe_e{oh}")
                nc.vector.tensor_add(t[:], wpwvd[oh][:], vecs["bpe"][oh][:])
                bp_e.append(t)

            for nm in ("q", "k", "v"):
                for h in range(2):
                    nc.scalar.activation(
                        w_t[nm][h][:], w_t[nm][h][:], AF.Copy,
                        scale=a_t[h][:])

            # ---------------- projections (read x16 directly) -------------
            # k first (gates chunk-0 scores), then q, then v behind the
            # first score matmuls. All attention operands land as fp8.
            q8 = big.tile([128, 2, M], F8, name="q8")
            k8 = big.tile([128, 2, L], F8, name="k8")
            vT8 = big.tile([128, NJP, 2, 256], F8, name="vT8")

            for oh in range(2):
                for it in range(L // 512):
                    ps = ps_s.tile([128, 512], F32, tag="s", name="ps_k")
                    for ch in range(2):
                        nc.tensor.matmul(
                            ps[:],
                            w_t["k"][ch][:, oh * 128:(oh + 1) * 128],
                            x16_t[ch][:, it * 512:(it + 1) * 512],
                            start=(ch == 0), stop=(ch == 1),
                        )
                    # softmax shift-invariance: k needs no bias
                    nc.vector.tensor_copy(
                        k8[:, oh, it * 512:(it + 1) * 512], ps[:])

            for oh in range(2):
                for it in range(M // 512):
                    ps = ps_s.tile([128, 512], F32, tag="s", name="ps_q")
                    for ch in range(2):
                        nc.tensor.matmul(
                            ps[:],
                            w_t["q"][ch][:, oh * 128:(oh + 1) * 128],
                            x16_t[ch][:, it * 512:(it + 1) * 512],
                            start=(ch == 0), stop=(ch == 1),
                        )
                    nc.vector.tensor_scalar_add(
                        q8[:, oh, it * 512:(it + 1) * 512], ps[:],
                        bq_e[oh][:])

            # ---------------- attention, chunk by chunk ----------------
            chunks = [(0, 512), (512, 512), (1024, 512),
                      (1536, 256), (1792, 256)]

            def emit_scores(cn, i0, chw):
                pT = ptp.tile([128, NJT, 512], F8, tag="pT", name=f"pT{cn}")
                for jp in range(NJP):
                    ps3 = ps_s.tile([128, 2, 512], F32, tag="s",
                                    name="ps_sc")
                    for half in range(2):
                        jt = jp * 2 + half
                        nc.tensor.matmul(
                            ps3[:, half, 0:chw],
                            k8[:, :, jt * 128:(jt + 1) * 128],
                            q8[:, :, i0:i0 + chw],
                            start=True, stop=True, perf_mode=DR,
                        )
                    nc.scalar.activation(
                        pT[:, jp * 2:jp * 2 + 2, 0:chw], ps3[:, :, 0:chw],
                        AF.Exp, scale=SCALE, bias=csh[:])
                return pT

            def emit_av(cn, i0, chw, pT):
                ps_av = [ps_acc.tile([128, chw], F32, tag=f"av{ch}",
                                     name=f"av{ch}_{cn}") for ch in range(2)]
                ps_den = ps_acc.tile([128, chw], F32, tag="den",
                                     name=f"den{cn}")
                for jp in range(NJP):
                    pslice = pT[:, jp * 2:jp * 2 + 2, 0:chw]
                    for ch in range(2):
                        nc.tensor.matmul(
                            ps_av[ch][:],
                            vT8[:, jp, :, ch * 128:(ch + 1) * 128],
                            pslice,
                            start=(jp == 0), stop=(jp == NJP - 1),
                            perf_mode=DR,
                        )
                    nc.tensor.matmul(
                        ps_den[:], ones8[:], pslice,
                        start=(jp == 0), stop=(jp == NJP - 1),
                        perf_mode=DR,
                    )

                rec = epi.tile([128, chw], F32, tag="rec", name=f"rec{cn}")
                nc.vector.reciprocal_approx_fast(rec[:], ps_den[:])

                at_t = []
                for ch in range(2):
                    at = epi.tile([128, chw], BF16, tag=f"at{ch}",
                                  name=f"at{ch}_{cn}")
                    nc.vector.tensor_mul(at[:], ps_av[ch][:], rec[:])
                    at_t.append(at)

                for oh in range(2):
                    ps = ps_o.tile([128, chw], F32, tag="o", name=f"po{oh}_{cn}")
                    for ch in range(2):
                        nc.tensor.matmul(
                            ps[:],
                            w_t["p"][ch][:, oh * 128:(oh + 1) * 128],
                            at_t[ch][:],
                            start=(ch == 0), stop=(ch == 1),
                        )
                    res = epi.tile([128, chw], F32, tag="res",
                                   name=f"res{oh}_{cn}")
                    nc.vector.scalar_tensor_tensor(
                        out=res[:], in0=ps[:], scalar=bp_e[oh][:],
                        in1=x_t[oh][:, i0:i0 + chw],
                        op0=mybir.AluOpType.add, op1=mybir.AluOpType.add,
                    )
                    nc.sync.dma_start(
                        out_d[oh * 128:(oh + 1) * 128, i0:i0 + chw], res[:])

            # chunk-0 scores before the v projection: ACT exp starts while
            # the PE fills v
            pT0 = emit_scores(0, chunks[0][0], chunks[0][1])

            for jp in range(NJP):
                # v PSUM alternates the av0/av1 banks (idle until chunk-0
                # AV, which needs vT8 complete anyway)
                psv = ps_acc.tile([128, 2, 256], F32, tag=f"av{jp % 2}",
                                  name="ps_v")
                for half in range(2):
                    lt = jp * 2 + half
                    for ch in range(2):
                        nc.tensor.matmul(
                            psv[:, half, :],
                            x16_t[ch][:, lt * 128:(lt + 1) * 128],
                            w_t["v"][ch][:],
                            start=(ch == 0), stop=(ch == 1),
                        )
                nc.vector.tensor_copy(vT8[:, jp, :, :], psv[:])

            prev = (0, chunks[0][0], chunks[0][1], pT0)
            for cn in range(1, len(chunks)):
                i0, chw = chunks[cn]
                pT = emit_scores(cn, i0, chw)
                emit_av(*prev)
                prev = (cn, i0, chw, pT)
            emit_av(*prev)

    nc.compile()
    return nc


def kernel(x, gamma, beta, wq, bq, wk, bk, wv, bv, wp, bp):
    global _COMPILED, LAST_EXEC_NS
    x = np.asarray(x, np.float32)
    if _COMPILED is None:
        _COMPILED = _build()
    nc = _COMPILED

    common = {
        "wqT": np.ascontiguousarray(np.asarray(wq, np.float32).T).astype(ml_dtypes.bfloat16),
        "wkT": np.ascontiguousarray(np.asarray(wk, np.float32).T).astype(ml_dtypes.bfloat16),
        "wvT": np.ascontiguousarray(np.asarray(wv, np.float32).T).astype(ml_dtypes.bfloat16),
        "wpT": np.ascontiguousarray(np.asarray(wp, np.float32).T).astype(ml_dtypes.bfloat16),
        "bq": np.asarray(bq, np.float32).reshape(C, 1),
        "bpe": (np.asarray(bp, np.float32)
                + np.asarray(wp, np.float32) @ np.asarray(bv, np.float32)
                ).reshape(C, 1),
        "gamma": np.asarray(gamma, np.float32).reshape(C, 1),
        "beta": np.asarray(beta, np.float32).reshape(C, 1),
    }

    x8 = [np.ascontiguousarray(x[b]).astype(ml_dtypes.float8_e4m3)
          for b in range(B)]

    in_maps = []
    for core in range(N_CORES):
        b, qh = core // 2, core % 2
        xb = x[b]
        if qh:
            xb = np.ascontiguousarray(np.roll(xb, -M, axis=1))
        others = np.concatenate([x8[s] for s in range(B) if s != b])
        in_maps.append({"x": xb, "x16": xb.astype(ml_dtypes.bfloat16),
                        "xs": others, **common})

    trace = os.environ.get("BASS_KERNEL_TRACE", "") == "1"
    res = bass_utils.run_bass_kernel_spmd(
        nc, in_maps, core_ids=list(range(N_CORES)), trace=trace)
    LAST_EXEC_NS = res.exec_time_ns

    out = np.empty((B, C, L), np.float32)
    for core in range(N_CORES):
        b, qh = core // 2, core % 2
        out[b, :, qh * M:(qh + 1) * M] = res.results[core]["out"]
    return out


# revision 7
# speedup vs baseline: 1.6253x; 1.1207x over previous
"""AttnBlock1D (BN + single-head 1x1-conv attention + residual) on 8 TRN2 cores.

Contract: kernel(**inputs) takes the FULL inputs from setup_inputs() and
returns the FULL output [4, 256, 4096] f32.

Sharding: 8 cores = 4 samples x 2 query-halves (data-parallel over B,
attention split over queries). Core i handles sample b = i // 2 and
queries [qh*2048, (qh+1)*2048), qh = i % 2. The host rolls x[b] along L
so each core's queries are the FIRST 2048 columns -- attention is
permutation-invariant over keys, so k/v built from the rolled layout give
identical softmax results; the SPMD program needs no per-core constants.

BatchNorm stats are computed locally on every core -- NO collective (any
cross-core sync puts the NEFF start skew onto the measured span) -- and
entirely ON THE PE from a host-transposed fp8 copy xT8 ([l, c] layout,
keys on partitions, shared by all cores): per 256-key pair, DoubleRow
matmuls accumulate the per-channel-block Gram diagonals (sum x^2) and a
ones-stationary row-sum (sum x, which lands on the diagonal of its own
PSUM block too). One tensor_tensor_reduce per block against an identity
mask extracts the diagonals. This keeps the whole ~45us DVE/ACT stats
phase off the startup critical path: startup is DMA-bound, and the Gram
matmuls double as PE warm-up. rsqrt(var+eps) is exp(-0.5*ln(var+eps)) so
every ACT function used (Copy/Square-free now: Copy/Ln/Exp) lives in the
single natural_log_exp_and_others table -- zero mid-kernel table loads.

The BN affine folds into the projections: effective biases via N=1
matvecs on the raw bf16 weights, then w8 = fp8(w * a[c]) per input
channel (ACT Copy with per-partition scale, fp8 out). The k bias drops
entirely (softmax shift-invariance); the v-path constant folds into the
output projection bias.

Everything heavy runs in fp8-e4m3 DoubleRow (contract 256 per
instruction, ~2.2x bf16 measured): q/k/v projections read x8 (fp8 of the
rolled sample) against w8; scores per key tile are one DoubleRow matmul;
exp on ACT (scale=1/16, bias=-3 folded in -- max scaled score ~8
overflows e4m3's 448 unshifted; softmax cancels the shift) writes fp8
probabilities keys-on-partitions; AV, the ones-matmul denominator, and
the output projection (host-quantized wp8) all contract DoubleRow.
reciprocal_approx_fast + one tensor_mul per channel-half normalizes out
of PSUM. Queries run in 5 chunks (3x512 + 2x256), double-buffered pT;
chunk-0 scores are emitted before the v projection so ACT exp starts as
early as possible. The f32 x arrives last and feeds only the residual.
"""

import os

import numpy as np
import ml_dtypes

import concourse.bass as bass
import concourse.mybir as mybir
import concourse.tile as tile
from concourse import bacc
from concourse import bass_utils

F32 = mybir.dt.float32
BF16 = mybir.dt.bfloat16
F8 = mybir.dt.float8e4
DR = mybir.MatmulPerfMode.DoubleRow

N_CORES = 8
B, C, L = 4, 256, 4096
M = L // 2          # queries per core
EPS = 1e-5
SCALE = 1.0 / 16.0  # C ** -0.5
CSHIFT = 3.0        # exp bias: p = exp(s/16 - CSHIFT); cancels in softmax

NJT = L // 128      # 32 key tiles
NJP = NJT // 2      # 16 key-tile pairs (DoubleRow contracts 256 keys)
NPAIR = B * NJP     # 64 stat pairs across the batch
NSTCH = 8           # xT8 arrives in 8 chunks of 8 pairs
AF = mybir.ActivationFunctionType

LAST_EXEC_NS = None
_COMPILED = None


def _build():
    nc = bacc.Bacc("TRN2", target_bir_lowering=False, debug=False,
                   num_devices=N_CORES)

    x_d = nc.dram_tensor("x", [C, L], F32, kind="ExternalInput")
    xT8_d = nc.dram_tensor("xT8", [128, NPAIR * 512], F8, kind="ExternalInput")
    x8_d = nc.dram_tensor("x8", [C, L], F8, kind="ExternalInput")
    wq_d = nc.dram_tensor("wqT", [C, C], BF16, kind="ExternalInput")
    wk_d = nc.dram_tensor("wkT", [C, C], BF16, kind="ExternalInput")
    wv_d = nc.dram_tensor("wvT", [C, C], BF16, kind="ExternalInput")
    wp_d = nc.dram_tensor("wpT", [C, C], BF16, kind="ExternalInput")
    wp8_d = nc.dram_tensor("wp8", [128, 2 * C], F8, kind="ExternalInput")
    id_d = nc.dram_tensor("id128", [128, 128], BF16, kind="ExternalInput")
    bq_d = nc.dram_tensor("bq", [C, 1], F32, kind="ExternalInput")
    bp_d = nc.dram_tensor("bpe", [C, 1], F32, kind="ExternalInput")
    gam_d = nc.dram_tensor("gamma", [C, 1], F32, kind="ExternalInput")
    bet_d = nc.dram_tensor("beta", [C, 1], F32, kind="ExternalInput")
    out_d = nc.dram_tensor("out", [C, M], F32, kind="ExternalOutput")

    with tile.TileContext(nc) as tc:
        with (
            tc.tile_pool(name="big", bufs=1) as big,
            tc.tile_pool(name="pt", bufs=2) as ptp,
            tc.tile_pool(name="small", bufs=2) as sm,
            tc.tile_pool(name="eps", bufs=3) as epi,
            tc.tile_pool(name="ps_s", bufs=2, space="PSUM") as ps_s,
            tc.tile_pool(name="ps_acc", bufs=1, space="PSUM") as ps_acc,
            tc.tile_pool(name="ps_o", bufs=1, space="PSUM") as ps_o,
        ):
            ones8 = big.tile([128, 2, 128], F8, name="ones8")
            nc.vector.memset(ones8[:], 1.0)
            csh = big.tile([128, 1], F32, name="csh")
            nc.vector.memset(csh[:], -CSHIFT)

            id_t = big.tile([128, 128], BF16, name="id_t")
            nc.sync.dma_start(id_t[:], id_d[:, :])

            vecs = {}
            for nm, d in (("bq", bq_d), ("bpe", bp_d),
                          ("gam", gam_d), ("bet", bet_d)):
                vecs[nm] = [big.tile([128, 1], F32, name=f"{nm}{h}")
                            for h in range(2)]
                for h in range(2):
                    nc.sync.dma_start(vecs[nm][h][:],
                                      d[h * 128:(h + 1) * 128, :])

            # ------- BN stats on the PE: Gram diagonals + row sums --------
            # xT8 chunk tiles [128, 8, 2, 256]: (p, jp_local, i, c) holds
            # x8[c, jp*256 + i*128 + p] summed over all 4 samples' length.
            g_ps = [ps_acc.tile([128, 128], F32, tag=f"av{h}", name=f"g{h}")
                    for h in range(2)]
            m_ps = ps_acc.tile([128, C], F32, tag="den", name="m_ps")
            PPC = NPAIR // NSTCH     # pairs per chunk
            for t in range(NSTCH):
                xst = sm.tile([128, PPC, 2, 256], F8, tag="xst", bufs=3,
                              name=f"xst{t}")
                nc.sync.dma_start(
                    xst[:], xT8_d[:, t * PPC * 512:(t + 1) * PPC * 512])
                for jpl in range(PPC):
                    jp = t * PPC + jpl
                    first, last = jp == 0, jp == NPAIR - 1
                    for h in range(2):
                        nc.tensor.matmul(
                            g_ps[h][:],
                            xst[:, jpl, :, h * 128:(h + 1) * 128],
                            xst[:, jpl, :, h * 128:(h + 1) * 128],
                            start=first, stop=last, perf_mode=DR,
                        )
                    nc.tensor.matmul(
                        m_ps[:], ones8[:], xst[:, jpl, :, :],
                        start=first, stop=last, perf_mode=DR,
                    )

            # weights + x8 stream in behind the stats input
            w_t = {}
            for nm, d in (("q", wq_d), ("k", wk_d), ("v", wv_d), ("p", wp_d)):
                w_t[nm] = [big.tile([128, C], BF16, name=f"w{nm}{h}")
                           for h in range(2)]
                for h in range(2):
                    nc.sync.dma_start(w_t[nm][h][:],
                                      d[h * 128:(h + 1) * 128, :])
            x8 = big.tile([128, 2, L], F8, name="x8")
            for h in range(2):
                nc.sync.dma_start(x8[:, h, :], x8_d[h * 128:(h + 1) * 128, :])
            wp8 = big.tile([128, 2, C], F8, name="wp8t")
            for ch in range(2):
                nc.sync.dma_start(wp8[:, ch, :], wp8_d[:, ch * C:(ch + 1) * C])

            # f32 x arrives late; only the epilogue residual reads it
            x_t = [big.tile([128, L], F32, name=f"x{h}") for h in range(2)]
            for h in range(2):
                nc.sync.dma_start(x_t[h][:], x_d[h * 128:(h + 1) * 128, :])

            # ------- extract diagonals, combine -> a (scale), d (shift) ---
            NT = B * L
            a_t, d_t = [], []
            for h in range(2):
                scr = sm.tile([128, 128], F32, tag="scr", bufs=4,
                              name=f"scrg{h}")
                sq = sm.tile([128, 1], F32, name=f"sq{h}")
                nc.vector.tensor_mul(scr[:], g_ps[h][:], id_t[:])
                nc.vector.reduce_sum(sq[:], scr[:],
                                     axis=mybir.AxisListType.X)
                scrm = sm.tile([128, 128], F32, tag="scr", bufs=4,
                               name=f"scrm{h}")
                mn = sm.tile([128, 1], F32, name=f"mn{h}")
                nc.vector.tensor_mul(scrm[:], m_ps[:, h * 128:(h + 1) * 128],
                                     id_t[:])
                nc.vector.reduce_sum(mn[:], scrm[:],
                                     axis=mybir.AxisListType.X)
                # ngm = -mean;  ge2p = E[x^2] + EPS;  var = ge2p - ngm^2
                ngm = sm.tile([128, 1], F32, name=f"ngm{h}")
                nc.vector.tensor_scalar_mul(ngm[:], mn[:], -1.0 / NT)
                ge2p = sm.tile([128, 1], F32, name=f"ge2p{h}")
                nc.vector.tensor_scalar(
                    out=ge2p[:], in0=sq[:], scalar1=1.0 / NT, scalar2=EPS,
                    op0=mybir.AluOpType.mult, op1=mybir.AluOpType.add)
                var = sm.tile([128, 1], F32, name=f"var{h}")
                nc.vector.scalar_tensor_tensor(
                    out=var[:], in0=ngm[:], scalar=ngm[:], in1=ge2p[:],
                    op0=mybir.AluOpType.mult, op1=mybir.AluOpType.subtract)
                nc.vector.tensor_scalar_mul(var[:], var[:], -1.0)
                # rs = rsqrt(var) = exp(-0.5 * ln(var)): stays in the one
                # ACT table (natural_log_exp_and_others), no table switch
                lnv = sm.tile([128, 1], F32, name=f"lnv{h}")
                nc.scalar.activation(lnv[:], var[:], AF.Ln)
                rs = sm.tile([128, 1], F32, name=f"rs{h}")
                nc.scalar.activation(rs[:], lnv[:], AF.Exp, scale=-0.5)
                a = sm.tile([128, 1], F32, name=f"a{h}")
                nc.vector.tensor_mul(a[:], rs[:], vecs["gam"][h][:])
                dd = sm.tile([128, 1], F32, name=f"d{h}")
                nc.vector.scalar_tensor_tensor(
                    out=dd[:], in0=a[:], scalar=ngm[:], in1=vecs["bet"][h][:],
                    op0=mybir.AluOpType.mult, op1=mybir.AluOpType.add)
                a_t.append(a)
                d_t.append(dd)

            # ------- fold BN affine into weights + effective biases -------
            # b*_eff = w @ d + b uses the RAW weights (tiny matvecs), then
            # w8 = fp8(w[c, o] * a[c]).
            d16 = [sm.tile([128, 1], BF16, name=f"d16_{h}") for h in range(2)]
            for h in range(2):
                nc.vector.tensor_copy(d16[h][:], d_t[h][:])

            def matvec(wtiles, rhs16, name):
                """out[o] = sum_c w[o, c] * rhs[c] as [2][128, 1] sbuf f32"""
                outs = []
                for oh in range(2):
                    ps = ps_s.tile([128, 1], F32, tag="s", name=f"mv_{name}{oh}")
                    for ch in range(2):
                        nc.tensor.matmul(
                            ps[:],
                            wtiles[ch][:, oh * 128:(oh + 1) * 128],
                            rhs16[ch][:],
                            start=(ch == 0), stop=(ch == 1),
                        )
                    o = sm.tile([128, 1], F32, name=f"mvo_{name}{oh}")
                    nc.vector.tensor_copy(o[:], ps[:])
                    outs.append(o)
                return outs

            wqd = matvec(w_t["q"], d16, "q")
            wvd = matvec(w_t["v"], d16, "v")
            bq_e = []
            for oh in range(2):
                t = sm.tile([128, 1], F32, name=f"bqe{oh}")
                nc.vector.tensor_add(t[:], wqd[oh][:], vecs["bq"][oh][:])
                bq_e.append(t)
            # bpe_eff = bpe + wp @ (wv @ d)
            wvd16 = [sm.tile([128, 1], BF16, name=f"wvd16_{h}")
                     for h in range(2)]
            for h in range(2):
                nc.vector.tensor_copy(wvd16[h][:], wvd[h][:])
            wpwvd = matvec(w_t["p"], wvd16, "p")
            bp_e = []
            for oh in range(2):
                t = sm.tile([128, 1], F32, name=f"bpe_e{oh}")
                nc.vector.tensor_add(t[:], wpwvd[oh][:], vecs["bpe"][oh][:])
                bp_e.append(t)

            w8 = {}
            for nm in ("q", "k", "v"):
                w8[nm] = big.tile([128, 2, C], F8, name=f"w8{nm}")
                for ch in range(2):
                    nc.scalar.activation(
                        w8[nm][:, ch, :], w_t[nm][ch][:], AF.Copy,
                        scale=a_t[ch][:])

            # ---------------- projections (fp8 DoubleRow) -----------------
            # k first (gates chunk-0 scores), then q, then v behind the
            # first score matmuls.
            q8 = big.tile([128, 2, M], F8, name="q8")
            k8 = big.tile([128, 2, L], F8, name="k8")
            vT8 = big.tile([128, NJP, 2, 256], F8, name="vT8")

            for oh in range(2):
                for it in range(L // 512):
                    ps = ps_s.tile([128, 512], F32, tag="s", name="ps_k")
                    nc.tensor.matmul(
                        ps[:],
                        w8["k"][:, :, oh * 128:(oh + 1) * 128],
                        x8[:, :, it * 512:(it + 1) * 512],
                        start=True, stop=True, perf_mode=DR,
                    )
                    # softmax shift-invariance: k needs no bias
                    nc.vector.tensor_copy(
                        k8[:, oh, it * 512:(it + 1) * 512], ps[:])

            for oh in range(2):
                for it in range(M // 512):
                    ps = ps_s.tile([128, 512], F32, tag="s", name="ps_q")
                    nc.tensor.matmul(
                        ps[:],
                        w8["q"][:, :, oh * 128:(oh + 1) * 128],
                        x8[:, :, it * 512:(it + 1) * 512],
                        start=True, stop=True, perf_mode=DR,
                    )
                    nc.vector.tensor_scalar_add(
                        q8[:, oh, it * 512:(it + 1) * 512], ps[:],
                        bq_e[oh][:])

            # ---------------- attention, chunk by chunk ----------------
            chunks = [(0, 512), (512, 512), (1024, 512),
                      (1536, 256), (1792, 256)]

            def emit_scores(cn, i0, chw):
                pT = ptp.tile([128, NJT, 512], F8, tag="pT", name=f"pT{cn}")
                for jp in range(NJP):
                    ps3 = ps_s.tile([128, 2, 512], F32, tag="s",
                                    name="ps_sc")
                    for half in range(2):
                        jt = jp * 2 + half
                        nc.tensor.matmul(
                            ps3[:, half, 0:chw],
                            k8[:, :, jt * 128:(jt + 1) * 128],
                            q8[:, :, i0:i0 + chw],
                            start=True, stop=True, perf_mode=DR,
                        )
                    nc.scalar.activation(
                        pT[:, jp * 2:jp * 2 + 2, 0:chw], ps3[:, :, 0:chw],
                        AF.Exp, scale=SCALE, bias=csh[:])
                return pT

            def emit_av(cn, i0, chw, pT):
                ps_av = [ps_acc.tile([128, chw], F32, tag=f"av{ch}",
                                     name=f"av{ch}_{cn}") for ch in range(2)]
                ps_den = ps_acc.tile([128, chw], F32, tag="den",
                                     name=f"den{cn}")
                for jp in range(NJP):
                    pslice = pT[:, jp * 2:jp * 2 + 2, 0:chw]
                    for ch in range(2):
                        nc.tensor.matmul(
                            ps_av[ch][:],
                            vT8[:, jp, :, ch * 128:(ch + 1) * 128],
                            pslice,
                            start=(jp == 0), stop=(jp == NJP - 1),
                            perf_mode=DR,
                        )
                    nc.tensor.matmul(
                        ps_den[:], ones8[:], pslice,
                        start=(jp == 0), stop=(jp == NJP - 1),
                        perf_mode=DR,
                    )

                rec = epi.tile([128, chw], F32, tag="rec", name=f"rec{cn}")
                nc.vector.reciprocal_approx_fast(rec[:], ps_den[:])

                at3 = epi.tile([128, 2, 512], F8, tag="at", name=f"at{cn}")
                for ch in range(2):
                    nc.vector.tensor_mul(
                        at3[:, ch, 0:chw], ps_av[ch][:], rec[:])

                for oh in range(2):
                    ps = ps_o.tile([128, chw], F32, tag="o", name=f"po{oh}_{cn}")
                    nc.tensor.matmul(
                        ps[:],
                        wp8[:, :, oh * 128:(oh + 1) * 128],
                        at3[:, :, 0:chw],
                        start=True, stop=True, perf_mode=DR,
                    )
                    res = epi.tile([128, chw], F32, tag="res",
                                   name=f"res{oh}_{cn}")
                    nc.vector.scalar_tensor_tensor(
                        out=res[:], in0=ps[:], scalar=bp_e[oh][:],
                        in1=x_t[oh][:, i0:i0 + chw],
                        op0=mybir.AluOpType.add, op1=mybir.AluOpType.add,
                    )
                    nc.sync.dma_start(
                        out_d[oh * 128:(oh + 1) * 128, i0:i0 + chw], res[:])

            # chunk-0 scores before the v projection: ACT exp starts while
            # the PE fills v
            pT0 = emit_scores(0, chunks[0][0], chunks[0][1])

            for jp in range(NJP):
                # v PSUM alternates the av0/av1 banks (idle until chunk-0
                # AV, which needs vT8 complete anyway)
                psv = ps_acc.tile([128, 2, 256], F32, tag=f"av{jp % 2}",
                                  name="ps_v")
                for half in range(2):
                    lt = jp * 2 + half
                    nc.tensor.matmul(
                        psv[:, half, :],
                        x8[:, :, lt * 128:(lt + 1) * 128],
                        w8["v"][:],
                        start=True, stop=True, perf_mode=DR,
                    )
                nc.vector.tensor_copy(vT8[:, jp, :, :], psv[:])

            prev = (0, chunks[0][0], chunks[0][1], pT0)
            for cn in range(1, len(chunks)):
                i0, chw = chunks[cn]
                pT = emit_scores(cn, i0, chw)
                emit_av(*prev)
                prev = (cn, i0, chw, pT)
            emit_av(*prev)

    nc.compile()
    return nc


_XT8_CACHE = None


def kernel(x, gamma, beta, wq, bq, wk, bk, wv, bv, wp, bp):
    global _COMPILED, LAST_EXEC_NS, _XT8_CACHE
    x = np.asarray(x, np.float32)
    if _COMPILED is None:
        _COMPILED = _build()
    nc = _COMPILED

    wp32 = np.asarray(wp, np.float32)
    wpT = np.ascontiguousarray(wp32.T)
    # wp8[c_p, ch*C + o] = fp8(wpT[ch*128 + c_p, o])
    wp8 = np.ascontiguousarray(
        wpT.reshape(2, 128, C).transpose(1, 0, 2).reshape(128, 2 * C)
    ).astype(ml_dtypes.float8_e4m3)

    common = {
        "wqT": np.ascontiguousarray(np.asarray(wq, np.float32).T).astype(ml_dtypes.bfloat16),
        "wkT": np.ascontiguousarray(np.asarray(wk, np.float32).T).astype(ml_dtypes.bfloat16),
        "wvT": np.ascontiguousarray(np.asarray(wv, np.float32).T).astype(ml_dtypes.bfloat16),
        "wpT": wpT.astype(ml_dtypes.bfloat16),
        "wp8": wp8,
        "id128": np.eye(128, dtype=ml_dtypes.bfloat16),
        "bq": np.asarray(bq, np.float32).reshape(C, 1),
        "bpe": (np.asarray(bp, np.float32)
                + wp32 @ np.asarray(bv, np.float32)
                ).reshape(C, 1),
        "gamma": np.asarray(gamma, np.float32).reshape(C, 1),
        "beta": np.asarray(beta, np.float32).reshape(C, 1),
    }

    x8 = [np.ascontiguousarray(x[b]).astype(ml_dtypes.float8_e4m3)
          for b in range(B)]
    # xT8[p, ((b*16 + jp)*2 + i)*256 + c] = x8[b][c, jp*256 + i*128 + p]
    xT8 = np.concatenate(
        [x8[b].T.reshape(16, 2, 128, 256).transpose(2, 0, 1, 3)
         .reshape(128, 16 * 512) for b in range(B)], axis=1)
    xT8 = np.ascontiguousarray(xT8)

    in_maps = []
    for core in range(N_CORES):
        b, qh = core // 2, core % 2
        xb = x[b]
        if qh:
            xb = np.ascontiguousarray(np.roll(xb, -M, axis=1))
        in_maps.append({"x": xb, "x8": xb.astype(ml_dtypes.float8_e4m3),
                        "xT8": xT8, **common})

    trace = os.environ.get("BASS_KERNEL_TRACE", "") == "1"
    res = bass_utils.run_bass_kernel_spmd(
        nc, in_maps, core_ids=list(range(N_CORES)), trace=trace)
    LAST_EXEC_NS = res.exec_time_ns

    out = np.empty((B, C, L), np.float32)
    for core in range(N_CORES):
        b, qh = core // 2, core % 2
        out[b, :, qh * M:(qh + 1) * M] = res.results[core]["out"]
    return out
